# revision 20
# baseline (speedup 1.0000x reference)
"""Multi-head causal self-attention forward on 8 Trainium2 NeuronCores.

Problem: x[4,2048,1024] @ w_qkv[1024,3072] -> causal MHA (16 heads, d=64)
         -> @ w_out[1024,1024] + b_out.

Sharding: core c handles batch b = c//2 and head-group g = c%2 (8 heads).
Each core computes a partial output  attn_out_heads(g) @ w_out[rows(g)]
for its batch; host sums the two partials per batch (row-parallel out
projection) and adds b_out.

Per-core kernel (bf16 matmul inputs, fp32 PSUM accumulate), organized as
ONE global software pipeline so the PE never head-of-line blocks on
ScalarE exp results:

  - attention steps at (head-pair m, k-chunk ik, head-half, q-block qb)
    granularity: ST matmul [64-lane contraction] -> s_ps [128,512] ->
    ScalarE exp (scale=1/8) -> bf16 p tile -> PV matmul into u[65,512]
    (v has an interleaved ones-column for the softmax denominator).
    Causality via block skip + column clip + triangular-mask multiply
    on the diagonal block.
  - independent matmul work (QKV projections, V precompute, output
    projection) is chopped into 4-8 matmul "units" that are interleaved
    between attention steps by a deficit counter (ScalarE-time minus
    PE-time), with deadline forcing so every tile is ready before the
    attention step that consumes it.  This keeps the PE busy while
    ScalarE chews exp, and keeps ScalarE busy during projection work.
  - PSUM: 3 banks ST ring + 4 banks u accumulators + 1 bank filler ring.
  - input DMAs spread across the 3 DMA-capable queues (sync/gpsimd/
    scalar); w_qkv is concatenated per-core on the host so each
    contraction chunk is ONE descriptor; y output DMAs alternate
    sync/gpsimd.
  - normalization: reciprocal_approx_fast + gpsimd partition_broadcast
    + DVE multiply -> hd tiles; out = hd.T @ w_out accumulated over 4
    head-pair chunks -> y f32 (copies on DVE, not ScalarE).
"""

import sys

sys.path.insert(0, "/opt/trn_rl_repo")

from collections import deque

import numpy as np
import ml_dtypes

import concourse.bass as bass
import concourse.tile as tile
from concourse import bacc, mybir
from concourse.bass_utils import run_bass_kernel_spmd

BF16 = mybir.dt.bfloat16
F32 = mybir.dt.float32
NP_BF16 = ml_dtypes.bfloat16
EXP = mybir.ActivationFunctionType.Exp

B, T, C = 4, 2048, 1024
NCORES = 8
HC = 8  # heads per core
D = 64
DQ = HC * D  # 512
CA = C // 128  # 8 contraction chunks
NT128 = T // 128  # 16
SCALE = 1.0 / 8.0
BIG = 10**9

_cached = None


def _build():
    nc = bacc.Bacc("TRN2", target_bir_lowering=False, debug=False, num_devices=NCORES)

    xT = nc.dram_tensor("xT", [C, T], BF16, kind="ExternalInput")
    wqkv = nc.dram_tensor("wqkv", [C, 3 * DQ], BF16, kind="ExternalInput")
    # wo pre-reshaped on host to [128, 4*C]: col block j = w_out rows j*128..+128
    wo = nc.dram_tensor("wo", [128, 4 * C], BF16, kind="ExternalInput")
    trid = nc.dram_tensor("tri", [128, 128], BF16, kind="ExternalInput")
    y = nc.dram_tensor("y", [T, C], F32, kind="ExternalOutput")

    with tile.TileContext(nc) as tc:
        _emit(tc, nc, xT, wqkv, wo, trid, y)
    nc.compile()
    return nc


def _emit(tc, nc, xT, wqkv, wo, trid, y):
    from contextlib import ExitStack

    with ExitStack() as ctx:
        ep = ctx.enter_context

        persist = ep(tc.tile_pool(name="persist", bufs=1))
        qts = [persist.tile([128, T], BF16, tag=f"qt{m}", name=f"qt{m}") for m in range(4)]
        kts = [persist.tile([128, T], BF16, tag=f"kt{m}", name=f"kt{m}") for m in range(4)]
        vts = [persist.tile([128, HC * 65], BF16, tag=f"v{i}", name=f"v{i}") for i in range(NT128)]
        hds = [persist.tile([128, T], BF16, tag=f"hd{j}", name=f"hd{j}") for j in range(4)]
        wo_all = persist.tile([128, 4 * C], BF16, tag="wo", name="wo_all")
        tri = persist.tile([128, 128], BF16, tag="tri", name="tri")

        xin = ep(tc.tile_pool(name="xin", bufs=1))
        xts = [xin.tile([128, T], BF16, tag=f"x{a}", name=f"x{a}") for a in range(CA)]
        wsb = [
            xin.tile([128, 3 * DQ], BF16, tag=f"w{a}", name=f"wsb{a}") for a in range(CA)
        ]

        # PSUM: ST ring (2 banks) + u accumulators (4) + filler ring (2).
        sps = ep(tc.tile_pool(name="sps", bufs=2, space="PSUM"))
        fillp = ep(tc.tile_pool(name="fillp", bufs=2, space="PSUM"))
        u_ps = ep(tc.tile_pool(name="u_ps", bufs=4, space="PSUM"))
        p_pool = ep(tc.tile_pool(name="p_pool", bufs=12))
        norm = ep(tc.tile_pool(name="norm", bufs=4))

        # Input DMAs across the 3 DMA-capable queues, ordered so the
        # prolog-critical slices (x first token-half, wq, wk, tri) land
        # first on each queue and pace the chunk-major prolog.
        nc.scalar.dma_start(out=tri, in_=trid[:, :])
        for a in range(CA):
            sl = slice(a * 128, (a + 1) * 128)
            nc.sync.dma_start(out=xts[a][:, 0:1024], in_=xT[sl, 0:1024])
            nc.sync.dma_start(out=wsb[a][:, DQ : 2 * DQ], in_=wqkv[sl, DQ : 2 * DQ])
            nc.gpsimd.dma_start(out=wsb[a][:, 0:DQ], in_=wqkv[sl, 0:DQ])
            nc.scalar.dma_start(
                out=wsb[a][:, 2 * DQ : 3 * DQ], in_=wqkv[sl, 2 * DQ : 3 * DQ]
            )
        for a in range(CA):
            sl = slice(a * 128, (a + 1) * 128)
            nc.gpsimd.dma_start(out=xts[a][:, 1024:2048], in_=xT[sl, 1024:2048])
        nc.scalar.dma_start(out=wo_all, in_=wo[:, :])
        # The ones-columns of the interleaved [v|1] tiles never change.
        for tk in range(NT128):
            v_view = vts[tk].rearrange("p (h e) -> p h e", e=65)
            nc.vector.memset(v_view[:, :, 64:65], 1.0)

        # ---------------- filler units ----------------
        # Each unit: up to 8 independent matmuls accumulating into a
        # 1-bank psum + a finalize copy on DVE.  Units run strictly
        # sequentially on their psum ring (bufs=1).

        def _ps(d, name):
            if "t" not in d:
                d["t"] = d["pool"].tile([128, 512], F32, tag=d["tag"], name=name)
            return d["t"]

        def unit_v(tk):
            d = {"pool": fillp, "tag": "fill", "i": 0}

            def mk(a):
                def go():
                    nc.tensor.matmul(
                        _ps(d, "vps"),
                        xts[a][:, tk * 128 : (tk + 1) * 128],
                        wsb[a][:, 2 * DQ : 3 * DQ],
                        start=(a == 0),
                        stop=(a == CA - 1),
                        skip_group_check=True,
                    )

                return go

            def fin():
                v_view = vts[tk].rearrange("p (h e) -> p h e", e=65)
                nc.vector.tensor_copy(
                    v_view[:, :, 0:64], _ps(d, "vps").rearrange("p (h e) -> p h e", e=64)
                )

            d.update(mms=[mk(a) for a in range(CA)], fin=fin)
            return d

        def unit_qkt(m, off, dst, tbp, hb):
            col0 = tbp * 1024 + hb * 512
            d = {"pool": fillp, "tag": "fill", "i": 0}

            def mk(a):
                def go():
                    nc.tensor.matmul(
                        _ps(d, "qkps"),
                        wsb[a][:, off + m * 128 : off + (m + 1) * 128],
                        xts[a][:, col0 : col0 + 512],
                        start=(a == 0),
                        stop=(a == CA - 1),
                        skip_group_check=True,
                    )

                return go

            def fin():
                nc.vector.tensor_copy(dst[m][:, col0 : col0 + 512], _ps(d, "qkps"))

            d.update(mms=[mk(a) for a in range(CA)], fin=fin)
            return d

        def unit_out(tq, nb):
            d = {"pool": fillp, "tag": "fill", "i": 0}

            def mk(j):
                def go():
                    nc.tensor.matmul(
                        _ps(d, "ops"),
                        hds[j][:, tq * 128 : (tq + 1) * 128],
                        wo_all[:, j * C + nb * 512 : j * C + (nb + 1) * 512],
                        start=(j == 0),
                        stop=(j == 3),
                        skip_group_check=True,
                    )

                return go

            def fin():
                ob = norm.tile([128, 512], F32, tag="ob", name="ob")
                nc.vector.tensor_copy(ob, _ps(d, "ops"))
                k = tq * 2 + nb
                if d.get("drain"):
                    eng = (nc.sync, nc.gpsimd, nc.scalar)[k % 3]
                else:  # keep ScalarE's queue free for exp while attention runs
                    eng = (nc.sync, nc.gpsimd)[k % 2]
                eng.dma_start(
                    out=y[tq * 128 : (tq + 1) * 128, nb * 512 : (nb + 1) * 512],
                    in_=ob,
                )

            d.update(mms=[mk(j) for j in range(4)], fin=fin)
            return d

        # ---------------- attention steps ----------------
        # One step = (m, ik, qb) covering BOTH head-halves: the two ST
        # matmuls are emitted back-to-back into disjoint PE row groups
        # (tile_position) so they execute concurrently.
        steps = []
        for m in range(4):
            for qg, iks in ((0, range(8)), (1, range(16))):
                for ik in iks:
                    for qb in (2 * qg, 2 * qg + 1):
                        if 4 * qb + 3 < ik:
                            continue
                        steps.append((m, ik, qb))

        LAG = 4
        SLACK = 2

        # First step index needing each input region -> unit deadlines.
        first_need = {}
        for t, (m, ik, qb) in enumerate(steps):
            first_need.setdefault(("kt", m, ik // 8, (ik // 4) % 2), t)
            first_need.setdefault(("qt", m, qb // 2, qb % 2), t)
            first_need.setdefault(("v", ik), t)

        units = []
        for tk in range(NT128):
            d = unit_v(tk)
            d["deadline"] = first_need[("v", tk)] + LAG - 1  # consumed by PV, not ST
            units.append(d)
        for m in range(4):
            for tbp in range(2):
                for hb in range(2):
                    d = unit_qkt(m, 0, qts, tbp, hb)
                    d["deadline"] = first_need.get(("qt", m, tbp, hb), BIG)
                    units.append(d)
                    d = unit_qkt(m, DQ, kts, tbp, hb)
                    d["deadline"] = first_need.get(("kt", m, tbp, hb), BIG)
                    units.append(d)
        for tq in range(NT128):
            for nb in range(2):
                d = unit_out(tq, nb)
                d["deadline"] = BIG
                units.append(d)
        units.sort(key=lambda u: u["deadline"])
        uq = deque(units)
        cur = [None]

        def pump_one():
            if cur[0] is None:
                if not uq:
                    return False
                cur[0] = uq.popleft()
            u = cur[0]
            u["mms"][u["i"]]()
            u["i"] += 1
            if u["i"] == len(u["mms"]):
                u["fin"]()
                cur[0] = None
            return True

        def head_deadline():
            if cur[0] is not None:
                return cur[0]["deadline"]
            return uq[0]["deadline"] if uq else BIG

        # Prolog: the units due before the first step run chunk-major on
        # the (still unused) ST psum ring, so their matmuls interleave
        # with the serial arrival of the input DMA chunks.
        prolog = []
        while uq and uq[0]["deadline"] <= SLACK and len(prolog) < 3:
            d = uq.popleft()
            if len(prolog) < 2:
                d["pool"], d["tag"] = sps, "sps"
            prolog.append(d)
        for a in range(CA):
            for d in prolog:
                d["mms"][a]()
        for d in prolog:
            d["fin"]()

        # ---------------- step emitters ----------------
        staged = {}
        us = {}

        def emit_st_exp(t):
            m, ik, qb = steps[t]
            qstart = qb * 512
            c0 = min(max(128 * ik - qstart, 0), 512)
            kc = slice(ik * 128, (ik + 1) * 128)
            tiles = []
            for half in range(2):
                rq = slice(half * 64, half * 64 + 64)
                s_ps = sps.tile([128, 512], F32, tag="sps", name="sps")
                nc.tensor.matmul(
                    s_ps[:, c0:512],
                    kts[m][rq, kc],
                    qts[m][rq, qstart + c0 : qstart + 512],
                    start=True,
                    stop=True,
                    tile_position=(half * 64, 0),
                )
                tiles.append(s_ps)
            pts = []
            for half in range(2):
                p_t = p_pool.tile([128, 512], BF16, tag="p", name="pt")
                nc.scalar.activation(
                    p_t[:, c0:512], tiles[half][:, c0:512], EXP, scale=SCALE
                )
                if 0 <= 128 * ik - qstart < 512:  # diagonal block
                    nc.vector.tensor_mul(
                        p_t[:, c0 : c0 + 128], p_t[:, c0 : c0 + 128], tri
                    )
                pts.append(p_t)
            staged[t] = (pts, c0)
            return 512 - c0

        def emit_pv(t):
            m, ik, qb = steps[t]
            pts, c0 = staged.pop(t)
            for half in range(2):
                h = 2 * m + half
                key = (m, half, qb)
                if key not in us:
                    us[key] = u_ps.tile([65, 512], F32, tag="u", name=f"u{half}_{qb}")
                u = us[key]
                nc.tensor.matmul(
                    u[:, c0:512],
                    vts[ik][:, h * 65 : h * 65 + 65],
                    pts[half][:, c0:512],
                    start=(ik == 0),
                    stop=(ik == 4 * qb + 3),
                    skip_group_check=True,
                )
                if ik == 4 * qb + 3:
                    rq = slice(half * 64, half * 64 + 64)
                    rec_in = norm.tile([1, 512], F32, tag="ri", name="ri")
                    nc.vector.tensor_copy(rec_in, u[64:65, :])
                    rec = norm.tile([1, 512], F32, tag="rc", name="rc")
                    nc.vector.reciprocal_approx_fast(out=rec, in_=rec_in)
                    bc = norm.tile([64, 512], F32, tag="bc", name="bc")
                    nc.gpsimd.partition_broadcast(bc, rec)
                    nc.vector.tensor_mul(
                        hds[m][rq, qb * 512 : (qb + 1) * 512], u[0:64, :], bc
                    )
                    us.pop(key)

        # ---------------- the global pipeline ----------------
        deficit = 0.0
        for t in range(len(steps) + LAG):
            if t < len(steps):
                while head_deadline() <= t + SLACK:
                    if not pump_one():
                        break
                ncols = emit_st_exp(t)
                deficit += 2 * (160.0 + 0.68 * ncols) - (3 * ncols / 2.4 + 60.0)
                while deficit > 0 and pump_one():
                    deficit -= 228.0
            if t >= LAG:
                emit_pv(t - LAG)
        # Drain remaining units (output projection): alternate the psum
        # rings and emit the next unit's matmuls before the previous
        # unit's copy, so copies pipeline behind matmul groups.
        rem = []
        if cur[0] is not None:
            rem.append(cur[0])
            cur[0] = None
        rem.extend(uq)
        uq.clear()
        pend = deque()
        for k, dunit in enumerate(rem):
            dunit["drain"] = True
            if "t" not in dunit and k % 2 == 1:
                dunit["pool"], dunit["tag"] = sps, "sps"
            for f in dunit["mms"][dunit["i"] :]:
                f()
            dunit["i"] = len(dunit["mms"])
            pend.append(dunit)
            if len(pend) >= 2:
                pend.popleft()["fin"]()
        while pend:
            pend.popleft()["fin"]()


def _in_maps(x, w_qkv, w_out):
    maps = []
    for c in range(NCORES):
        b, g = c // 2, c % 2
        h0 = g * DQ
        wqkv = np.concatenate(
            [
                w_qkv[:, h0 : h0 + DQ],
                w_qkv[:, C + h0 : C + h0 + DQ],
                w_qkv[:, 2 * C + h0 : 2 * C + h0 + DQ],
            ],
            axis=1,
        )
        maps.append(
            {
                "xT": np.ascontiguousarray(x[b].T).astype(NP_BF16),
                "wqkv": np.ascontiguousarray(wqkv).astype(NP_BF16),
                "wo": np.ascontiguousarray(
                    w_out[h0 : h0 + DQ, :]
                    .reshape(4, 128, C)
                    .transpose(1, 0, 2)
                    .reshape(128, 4 * C)
                ).astype(NP_BF16),
                "tri": np.triu(np.ones((128, 128), dtype=np.float32)).astype(NP_BF16),
            }
        )
    return maps


def get_bass():
    global _cached
    if _cached is None:
        _cached = _build()
    return _cached


def run(x, w_qkv, w_out, b_out, **spmd_kwargs):
    nc = get_bass()
    res = run_bass_kernel_spmd(
        nc, _in_maps(x, w_qkv, w_out), core_ids=list(range(NCORES)), **spmd_kwargs
    )
    out = np.empty((B, T, C), dtype=np.float32)
    for b in range(B):
        out[b] = res.results[2 * b]["y"] + res.results[2 * b + 1]["y"]
    out += b_out.astype(np.float32)
    return out, res


def kernel(x, w_qkv, w_out, b_out):
    x = np.asarray(x)
    w_qkv = np.asarray(w_qkv)
    w_out = np.asarray(w_out)
    b_out = np.asarray(b_out)
    out, _ = run(x, w_qkv, w_out, b_out)
    return out


# revision 22
# speedup vs baseline: 1.1617x; 1.1617x over previous
"""Multi-head causal self-attention forward on 8 Trainium2 NeuronCores.

Problem: x[4,2048,1024] @ w_qkv[1024,3072] -> causal MHA (16 heads, d=64)
         -> @ w_out[1024,1024] + b_out.

Sharding: core c handles batch b = c//2 and head-group g = c%2 (8 heads).
Each core computes a partial output  attn_out_heads(g) @ w_out[rows(g)]
for its batch; host sums the two partials per batch (row-parallel out
projection) and adds b_out.

Per-core kernel (bf16 matmul inputs, fp32 PSUM accumulate), organized as
ONE global software pipeline so the PE never head-of-line blocks on
ScalarE exp results:

  - attention steps at (head-pair m, k-chunk ik, head-half, q-block qb)
    granularity: ST matmul [64-lane contraction] -> s_ps [128,512] ->
    ScalarE exp (scale=1/8) -> bf16 p tile -> PV matmul into u[65,512]
    (v has an interleaved ones-column for the softmax denominator).
    Causality via block skip + column clip + triangular-mask multiply
    on the diagonal block.
  - independent matmul work (QKV projections, V precompute, output
    projection) is chopped into 4-8 matmul "units" that are interleaved
    between attention steps by a deficit counter (ScalarE-time minus
    PE-time), with deadline forcing so every tile is ready before the
    attention step that consumes it.  This keeps the PE busy while
    ScalarE chews exp, and keeps ScalarE busy during projection work.
  - PSUM: 3 banks ST ring + 4 banks u accumulators + 1 bank filler ring.
  - input DMAs spread across the 3 DMA-capable queues (sync/gpsimd/
    scalar); w_qkv is concatenated per-core on the host so each
    contraction chunk is ONE descriptor; y output DMAs alternate
    sync/gpsimd.
  - normalization: reciprocal_approx_fast + gpsimd partition_broadcast
    + DVE multiply -> hd tiles; out = hd.T @ w_out accumulated over 4
    head-pair chunks -> y f32 (copies on DVE, not ScalarE).
"""

import sys

sys.path.insert(0, "/opt/trn_rl_repo")

from collections import deque

import numpy as np
import ml_dtypes

import concourse.bass as bass
import concourse.tile as tile
from concourse import bacc, mybir
from concourse.bass_utils import run_bass_kernel_spmd

BF16 = mybir.dt.bfloat16
F32 = mybir.dt.float32
NP_BF16 = ml_dtypes.bfloat16
EXP = mybir.ActivationFunctionType.Exp

B, T, C = 4, 2048, 1024
NCORES = 8
HC = 8  # heads per core
D = 64
DQ = HC * D  # 512
CA = C // 128  # 8 contraction chunks
NT128 = T // 128  # 16
SCALE = 1.0 / 8.0
BIG = 10**9

_cached = None


def _build():
    nc = bacc.Bacc("TRN2", target_bir_lowering=False, debug=False, num_devices=NCORES)

    xT = nc.dram_tensor("xT", [C, T], BF16, kind="ExternalInput")
    wqkv = nc.dram_tensor("wqkv", [C, 3 * DQ], BF16, kind="ExternalInput")
    # wo pre-reshaped on host to [128, 4*C]: col block j = w_out rows j*128..+128
    wo = nc.dram_tensor("wo", [128, 4 * C], BF16, kind="ExternalInput")
    trid = nc.dram_tensor("tri", [128, 128], BF16, kind="ExternalInput")
    y = nc.dram_tensor("y", [T, C], F32, kind="ExternalOutput")

    with tile.TileContext(nc) as tc:
        _emit(tc, nc, xT, wqkv, wo, trid, y)
    nc.compile()
    return nc


def _emit(tc, nc, xT, wqkv, wo, trid, y):
    from contextlib import ExitStack

    with ExitStack() as ctx:
        ep = ctx.enter_context

        persist = ep(tc.tile_pool(name="persist", bufs=1))
        qts = [persist.tile([128, T], BF16, tag=f"qt{m}", name=f"qt{m}") for m in range(4)]
        kts = [persist.tile([128, T], BF16, tag=f"kt{m}", name=f"kt{m}") for m in range(4)]
        vts = [persist.tile([128, HC * 65], BF16, tag=f"v{i}", name=f"v{i}") for i in range(NT128)]
        hds = [persist.tile([128, T], BF16, tag=f"hd{j}", name=f"hd{j}") for j in range(4)]
        wo_all = persist.tile([128, 4 * C], BF16, tag="wo", name="wo_all")
        tri = persist.tile([128, 128], BF16, tag="tri", name="tri")

        xin = ep(tc.tile_pool(name="xin", bufs=1))
        xts = [xin.tile([128, T], BF16, tag=f"x{a}", name=f"x{a}") for a in range(CA)]
        wsb = [
            xin.tile([128, 3 * DQ], BF16, tag=f"w{a}", name=f"wsb{a}") for a in range(CA)
        ]

        # PSUM: ST ring (2 banks) + u accumulators (4) + filler ring (2).
        sps = ep(tc.tile_pool(name="sps", bufs=2, space="PSUM"))
        fillp = ep(tc.tile_pool(name="fillp", bufs=2, space="PSUM"))
        u_ps = ep(tc.tile_pool(name="u_ps", bufs=4, space="PSUM"))
        p_pool = ep(tc.tile_pool(name="p_pool", bufs=12))
        norm = ep(tc.tile_pool(name="norm", bufs=4))

        # Input DMAs across the 3 DMA-capable queues, ordered so the
        # prolog-critical slices (x first token-half, wq, wk, tri) land
        # first on each queue and pace the chunk-major prolog.
        nc.scalar.dma_start(out=tri, in_=trid[:, :])
        for a in range(CA):
            sl = slice(a * 128, (a + 1) * 128)
            nc.sync.dma_start(out=xts[a][:, 0:1024], in_=xT[sl, 0:1024])
            nc.gpsimd.dma_start(out=wsb[a][:, 0:DQ], in_=wqkv[sl, 0:DQ])
            nc.scalar.dma_start(
                out=wsb[a][:, 2 * DQ : 3 * DQ], in_=wqkv[sl, 2 * DQ : 3 * DQ]
            )
        for a in range(CA):
            sl = slice(a * 128, (a + 1) * 128)
            nc.sync.dma_start(out=wsb[a][:, DQ : 2 * DQ], in_=wqkv[sl, DQ : 2 * DQ])
            nc.gpsimd.dma_start(out=xts[a][:, 1024:2048], in_=xT[sl, 1024:2048])
        nc.scalar.dma_start(out=wo_all, in_=wo[:, :])
        # The ones-columns of the interleaved [v|1] tiles never change.
        for tk in range(NT128):
            v_view = vts[tk].rearrange("p (h e) -> p h e", e=65)
            nc.vector.memset(v_view[:, :, 64:65], 1.0)

        # ---------------- filler units ----------------
        # Each unit: up to 8 independent matmuls accumulating into a
        # 1-bank psum + a finalize copy on DVE.  Units run strictly
        # sequentially on their psum ring (bufs=1).

        def _ps(d, name):
            if "t" not in d:
                d["t"] = d["pool"].tile([128, 512], F32, tag=d["tag"], name=name)
            return d["t"]

        def unit_v(tk):
            d = {"pool": fillp, "tag": "fill", "i": 0}

            def mk(a):
                def go():
                    nc.tensor.matmul(
                        _ps(d, "vps"),
                        xts[a][:, tk * 128 : (tk + 1) * 128],
                        wsb[a][:, 2 * DQ : 3 * DQ],
                        start=(a == 0),
                        stop=(a == CA - 1),
                        skip_group_check=True,
                    )

                return go

            def fin():
                v_view = vts[tk].rearrange("p (h e) -> p h e", e=65)
                nc.vector.tensor_copy(
                    v_view[:, :, 0:64], _ps(d, "vps").rearrange("p (h e) -> p h e", e=64)
                )

            d.update(mms=[mk(a) for a in range(CA)], fin=fin)
            return d

        def unit_qkt(m, off, dst, tbp, hb):
            col0 = tbp * 1024 + hb * 512
            d = {"pool": fillp, "tag": "fill", "i": 0}

            def mk(a):
                def go():
                    nc.tensor.matmul(
                        _ps(d, "qkps"),
                        wsb[a][:, off + m * 128 : off + (m + 1) * 128],
                        xts[a][:, col0 : col0 + 512],
                        start=(a == 0),
                        stop=(a == CA - 1),
                        skip_group_check=True,
                    )

                return go

            def fin():
                nc.vector.tensor_copy(dst[m][:, col0 : col0 + 512], _ps(d, "qkps"))

            d.update(mms=[mk(a) for a in range(CA)], fin=fin)
            return d

        def unit_out(tq, nb):
            d = {"pool": fillp, "tag": "fill", "i": 0}

            def mk(j):
                def go():
                    nc.tensor.matmul(
                        _ps(d, "ops"),
                        hds[j][:, tq * 128 : (tq + 1) * 128],
                        wo_all[:, j * C + nb * 512 : j * C + (nb + 1) * 512],
                        start=(j == 0),
                        stop=(j == 3),
                        skip_group_check=True,
                    )

                return go

            def fin():
                ob = norm.tile([128, 512], F32, tag="ob", name="ob")
                nc.vector.tensor_copy(ob, _ps(d, "ops"))
                eng = nc.sync if (tq * 2 + nb) % 2 == 0 else nc.gpsimd
                eng.dma_start(
                    out=y[tq * 128 : (tq + 1) * 128, nb * 512 : (nb + 1) * 512],
                    in_=ob,
                )

            d.update(mms=[mk(j) for j in range(4)], fin=fin)
            return d

        # ---------------- attention steps ----------------
        # One step = (m, ik, qb) covering BOTH head-halves: the two ST
        # matmuls are emitted back-to-back into disjoint PE row groups
        # (tile_position) so they execute concurrently.
        steps = []
        for m in range(4):
            for qg, iks in ((0, range(8)), (1, range(16))):
                for ik in iks:
                    for qb in (2 * qg, 2 * qg + 1):
                        if 4 * qb + 3 < ik:
                            continue
                        steps.append((m, ik, qb))

        LAG = 4
        SLACK = 2

        # First step index needing each input region -> unit deadlines.
        first_need = {}
        for t, (m, ik, qb) in enumerate(steps):
            first_need.setdefault(("kt", m, ik // 8, (ik // 4) % 2), t)
            first_need.setdefault(("qt", m, qb // 2, qb % 2), t)
            first_need.setdefault(("v", ik), t)

        units = []
        for tk in range(NT128):
            d = unit_v(tk)
            d["deadline"] = first_need[("v", tk)] + LAG - 1  # consumed by PV, not ST
            units.append(d)
        for m in range(4):
            for tbp in range(2):
                for hb in range(2):
                    d = unit_qkt(m, 0, qts, tbp, hb)
                    d["deadline"] = first_need.get(("qt", m, tbp, hb), BIG)
                    units.append(d)
                    d = unit_qkt(m, DQ, kts, tbp, hb)
                    d["deadline"] = first_need.get(("kt", m, tbp, hb), BIG)
                    units.append(d)
        for tq in range(NT128):
            for nb in range(2):
                d = unit_out(tq, nb)
                d["deadline"] = BIG
                units.append(d)
        units.sort(key=lambda u: u["deadline"])
        uq = deque(units)
        cur = [None]

        def pump_one():
            if cur[0] is None:
                if not uq:
                    return False
                cur[0] = uq.popleft()
            u = cur[0]
            u["mms"][u["i"]]()
            u["i"] += 1
            if u["i"] == len(u["mms"]):
                u["fin"]()
                cur[0] = None
            return True

        def head_deadline():
            if cur[0] is not None:
                return cur[0]["deadline"]
            return uq[0]["deadline"] if uq else BIG

        # Prolog: the units due before the first step run chunk-major on
        # the (still unused) ST psum ring, so their matmuls interleave
        # with the serial arrival of the input DMA chunks.
        prolog = []
        while uq and uq[0]["deadline"] <= SLACK and len(prolog) < 3:
            d = uq.popleft()
            if len(prolog) < 2:
                d["pool"], d["tag"] = sps, "sps"
            prolog.append(d)
        for a in range(CA):
            for d in prolog:
                d["mms"][a]()
        for d in prolog:
            d["fin"]()

        # ---------------- step emitters ----------------
        staged = {}
        us = {}

        def emit_st_exp(t):
            m, ik, qb = steps[t]
            qstart = qb * 512
            c0 = min(max(128 * ik - qstart, 0), 512)
            kc = slice(ik * 128, (ik + 1) * 128)
            tiles = []
            for half in range(2):
                rq = slice(half * 64, half * 64 + 64)
                s_ps = sps.tile([128, 512], F32, tag="sps", name="sps")
                nc.tensor.matmul(
                    s_ps[:, c0:512],
                    kts[m][rq, kc],
                    qts[m][rq, qstart + c0 : qstart + 512],
                    start=True,
                    stop=True,
                    tile_position=(half * 64, 0),
                )
                tiles.append(s_ps)
            pts = []
            for half in range(2):
                p_t = p_pool.tile([128, 512], BF16, tag="p", name="pt")
                nc.scalar.activation(
                    p_t[:, c0:512], tiles[half][:, c0:512], EXP, scale=SCALE
                )
                if 0 <= 128 * ik - qstart < 512:  # diagonal block
                    nc.vector.tensor_mul(
                        p_t[:, c0 : c0 + 128], p_t[:, c0 : c0 + 128], tri
                    )
                pts.append(p_t)
            staged[t] = (pts, c0)
            return 512 - c0

        def emit_pv(t):
            m, ik, qb = steps[t]
            pts, c0 = staged.pop(t)
            for half in range(2):
                h = 2 * m + half
                key = (m, half, qb)
                if key not in us:
                    us[key] = u_ps.tile([65, 512], F32, tag="u", name=f"u{half}_{qb}")
                u = us[key]
                nc.tensor.matmul(
                    u[:, c0:512],
                    vts[ik][:, h * 65 : h * 65 + 65],
                    pts[half][:, c0:512],
                    start=(ik == 0),
                    stop=(ik == 4 * qb + 3),
                    skip_group_check=True,
                )
                if ik == 4 * qb + 3:
                    rq = slice(half * 64, half * 64 + 64)
                    rec_in = norm.tile([1, 512], F32, tag="ri", name="ri")
                    nc.vector.tensor_copy(rec_in, u[64:65, :])
                    rec = norm.tile([1, 512], F32, tag="rc", name="rc")
                    nc.vector.reciprocal_approx_fast(out=rec, in_=rec_in)
                    bc = norm.tile([64, 512], F32, tag="bc", name="bc")
                    nc.gpsimd.partition_broadcast(bc, rec)
                    nc.vector.tensor_mul(
                        hds[m][rq, qb * 512 : (qb + 1) * 512], u[0:64, :], bc
                    )
                    us.pop(key)

        # ---------------- the global pipeline ----------------
        deficit = 0.0
        for t in range(len(steps) + LAG):
            if t < len(steps):
                while head_deadline() <= t + SLACK:
                    if not pump_one():
                        break
                ncols = emit_st_exp(t)
                deficit += 2 * (160.0 + 0.68 * ncols) - (3 * ncols / 2.4 + 60.0)
                while deficit > 0 and pump_one():
                    deficit -= 228.0
            if t >= LAG:
                emit_pv(t - LAG)
        # Drain remaining units (output projection): alternate the psum
        # rings and emit the next unit's matmuls before the previous
        # unit's copy, so copies pipeline behind matmul groups.
        rem = []
        if cur[0] is not None:
            rem.append(cur[0])
            cur[0] = None
        rem.extend(uq)
        uq.clear()
        pend = deque()
        for k, dunit in enumerate(rem):
            dunit["drain"] = True
            if "t" not in dunit and k % 2 == 1:
                dunit["pool"], dunit["tag"] = sps, "sps"
            for f in dunit["mms"][dunit["i"] :]:
                f()
            dunit["i"] = len(dunit["mms"])
            pend.append(dunit)
            if len(pend) >= 2:
                pend.popleft()["fin"]()
        while pend:
            pend.popleft()["fin"]()


def _in_maps(x, w_qkv, w_out):
    maps = []
    for c in range(NCORES):
        b, g = c // 2, c % 2
        h0 = g * DQ
        wqkv = np.concatenate(
            [
                w_qkv[:, h0 : h0 + DQ],
                w_qkv[:, C + h0 : C + h0 + DQ],
                w_qkv[:, 2 * C + h0 : 2 * C + h0 + DQ],
            ],
            axis=1,
        )
        maps.append(
            {
                "xT": np.ascontiguousarray(x[b].T).astype(NP_BF16),
                "wqkv": np.ascontiguousarray(wqkv).astype(NP_BF16),
                "wo": np.ascontiguousarray(
                    w_out[h0 : h0 + DQ, :]
                    .reshape(4, 128, C)
                    .transpose(1, 0, 2)
                    .reshape(128, 4 * C)
                ).astype(NP_BF16),
                "tri": np.triu(np.ones((128, 128), dtype=np.float32)).astype(NP_BF16),
            }
        )
    return maps


def get_bass():
    global _cached
    if _cached is None:
        _cached = _build()
    return _cached


def run(x, w_qkv, w_out, b_out, **spmd_kwargs):
    nc = get_bass()
    res = run_bass_kernel_spmd(
        nc, _in_maps(x, w_qkv, w_out), core_ids=list(range(NCORES)), **spmd_kwargs
    )
    out = np.empty((B, T, C), dtype=np.float32)
    for b in range(B):
        out[b] = res.results[2 * b]["y"] + res.results[2 * b + 1]["y"]
    out += b_out.astype(np.float32)
    return out, res


def kernel(x, w_qkv, w_out, b_out):
    x = np.asarray(x)
    w_qkv = np.asarray(w_qkv)
    w_out = np.asarray(w_out)
    b_out = np.asarray(b_out)
    out, _ = run(x, w_qkv, w_out, b_out)
    return out


# revision 27
# speedup vs baseline: 1.3311x; 1.1459x over previous
"""Multi-head causal self-attention forward on 8 Trainium2 NeuronCores.

Problem: x[4,2048,1024] @ w_qkv[1024,3072] -> causal MHA (16 heads, d=64)
         -> @ w_out[1024,1024] + b_out.

Sharding: core c handles batch b = c//2 and head-group g = c%2 (8 heads).
Each core computes a partial output  attn_out_heads(g) @ w_out[rows(g)]
for its batch; host sums the two partials per batch (row-parallel out
projection) and adds b_out.

Per-core kernel (bf16 matmul inputs, fp32 PSUM accumulate), organized as
ONE global software pipeline so the PE never head-of-line blocks on
ScalarE exp results:

  - attention steps at (head-pair m, k-chunk ik, head-half, q-block qb)
    granularity: ST matmul [64-lane contraction] -> s_ps [128,512] ->
    ScalarE exp (scale=1/8) -> bf16 p tile -> PV matmul into u[65,512]
    (v has an interleaved ones-column for the softmax denominator).
    Causality via block skip + column clip + triangular-mask multiply
    on the diagonal block.
  - independent matmul work (QKV projections, V precompute, output
    projection) is chopped into 4-8 matmul "units" that are interleaved
    between attention steps by a deficit counter (ScalarE-time minus
    PE-time), with deadline forcing so every tile is ready before the
    attention step that consumes it.  This keeps the PE busy while
    ScalarE chews exp, and keeps ScalarE busy during projection work.
  - PSUM: 3 banks ST ring + 4 banks u accumulators + 1 bank filler ring.
  - input DMAs spread across the 3 DMA-capable queues (sync/gpsimd/
    scalar); w_qkv is concatenated per-core on the host so each
    contraction chunk is ONE descriptor; y output DMAs alternate
    sync/gpsimd.
  - normalization: reciprocal_approx_fast + gpsimd partition_broadcast
    + DVE multiply -> hd tiles; out = hd.T @ w_out accumulated over 4
    head-pair chunks -> y f32 (copies on DVE, not ScalarE).
"""

import sys

sys.path.insert(0, "/opt/trn_rl_repo")

from collections import deque

import numpy as np
import ml_dtypes

import concourse.bass as bass
import concourse.tile as tile
from concourse import bacc, mybir
from concourse.bass_utils import run_bass_kernel_spmd

BF16 = mybir.dt.bfloat16
F32 = mybir.dt.float32
NP_BF16 = ml_dtypes.bfloat16
EXP = mybir.ActivationFunctionType.Exp

B, T, C = 4, 2048, 1024
NCORES = 8
HC = 8  # heads per core
D = 64
DQ = HC * D  # 512
CA = C // 128  # 8 contraction chunks
NT128 = T // 128  # 16
SCALE = 1.0 / 8.0
BIG = 10**9

_cached = None


def _build():
    nc = bacc.Bacc("TRN2", target_bir_lowering=False, debug=False, num_devices=NCORES)

    xT = nc.dram_tensor("xT", [C, T], BF16, kind="ExternalInput")
    wqkv = nc.dram_tensor("wqkv", [C, 3 * DQ], BF16, kind="ExternalInput")
    # wo pre-reshaped on host to [128, 4*C]: col block j = w_out rows j*128..+128
    wo = nc.dram_tensor("wo", [128, 4 * C], BF16, kind="ExternalInput")
    trid = nc.dram_tensor("tri", [128, 128], BF16, kind="ExternalInput")
    y = nc.dram_tensor("y", [T, C], F32, kind="ExternalOutput")

    with tile.TileContext(nc) as tc:
        _emit(tc, nc, xT, wqkv, wo, trid, y)
    nc.compile()
    return nc


def _emit(tc, nc, xT, wqkv, wo, trid, y):
    from contextlib import ExitStack

    with ExitStack() as ctx:
        ep = ctx.enter_context

        persist = ep(tc.tile_pool(name="persist", bufs=1))
        qts = [persist.tile([128, T], BF16, tag=f"qt{m}", name=f"qt{m}") for m in range(4)]
        kts = [persist.tile([128, T], BF16, tag=f"kt{m}", name=f"kt{m}") for m in range(4)]
        vts = [persist.tile([128, HC * 65], BF16, tag=f"v{i}", name=f"v{i}") for i in range(NT128)]
        hds = [persist.tile([128, T], BF16, tag=f"hd{j}", name=f"hd{j}") for j in range(4)]
        wo_all = persist.tile([128, 4 * C], BF16, tag="wo", name="wo_all")
        tri = persist.tile([128, 128], BF16, tag="tri", name="tri")

        xin = ep(tc.tile_pool(name="xin", bufs=1))
        xts = [xin.tile([128, T], BF16, tag=f"x{a}", name=f"x{a}") for a in range(CA)]
        wsb = [
            xin.tile([128, 3 * DQ], BF16, tag=f"w{a}", name=f"wsb{a}") for a in range(CA)
        ]

        # PSUM: ST ring (2 x 2 banks, both halves side by side) + u
        # accumulators (3, eagerly evacuated) + filler ring (1).
        sps = ep(tc.tile_pool(name="sps", bufs=2, space="PSUM"))
        fillp = ep(tc.tile_pool(name="fillp", bufs=1, space="PSUM"))
        u_ps = ep(tc.tile_pool(name="u_ps", bufs=3, space="PSUM"))
        p_pool = ep(tc.tile_pool(name="p_pool", bufs=12))
        norm = ep(tc.tile_pool(name="norm", bufs=4))

        # Input DMAs across the 3 DMA-capable queues, ordered so the
        # prolog-critical slices (x first token-half, wq, wk, tri) land
        # first on each queue and pace the chunk-major prolog.
        nc.scalar.dma_start(out=tri, in_=trid[:, :])
        for a in range(CA):
            sl = slice(a * 128, (a + 1) * 128)
            nc.sync.dma_start(out=xts[a][:, 0:1024], in_=xT[sl, 0:1024])
            nc.gpsimd.dma_start(out=wsb[a][:, 0:DQ], in_=wqkv[sl, 0:DQ])
            nc.scalar.dma_start(
                out=wsb[a][:, 2 * DQ : 3 * DQ], in_=wqkv[sl, 2 * DQ : 3 * DQ]
            )
        for a in range(CA):
            sl = slice(a * 128, (a + 1) * 128)
            nc.sync.dma_start(out=wsb[a][:, DQ : 2 * DQ], in_=wqkv[sl, DQ : 2 * DQ])
            nc.gpsimd.dma_start(out=xts[a][:, 1024:2048], in_=xT[sl, 1024:2048])
        nc.scalar.dma_start(out=wo_all, in_=wo[:, :])
        # The ones-columns of the interleaved [v|1] tiles never change.
        for tk in range(NT128):
            v_view = vts[tk].rearrange("p (h e) -> p h e", e=65)
            nc.vector.memset(v_view[:, :, 64:65], 1.0)

        # ---------------- filler units ----------------
        # Each unit: up to 8 independent matmuls accumulating into a
        # 1-bank psum + a finalize copy on DVE.  Units run strictly
        # sequentially on their psum ring (bufs=1).

        def _ps(d, name):
            if "t" not in d:
                d["t"] = d["pool"].tile([128, 512], F32, tag=d["tag"], name=name)
            return d["t"]

        def unit_v(tk):
            d = {"pool": fillp, "tag": "fill", "i": 0}

            def mk(a):
                def go():
                    nc.tensor.matmul(
                        _ps(d, "vps"),
                        xts[a][:, tk * 128 : (tk + 1) * 128],
                        wsb[a][:, 2 * DQ : 3 * DQ],
                        start=(a == 0),
                        stop=(a == CA - 1),
                        skip_group_check=True,
                    )

                return go

            def fin():
                v_view = vts[tk].rearrange("p (h e) -> p h e", e=65)
                nc.vector.tensor_copy(
                    v_view[:, :, 0:64], _ps(d, "vps").rearrange("p (h e) -> p h e", e=64)
                )

            d.update(mms=[mk(a) for a in range(CA)], fin=fin)
            return d

        def unit_qkt(m, off, dst, tbp, hb):
            col0 = tbp * 1024 + hb * 512
            d = {"pool": fillp, "tag": "fill", "i": 0}

            def mk(a):
                def go():
                    nc.tensor.matmul(
                        _ps(d, "qkps"),
                        wsb[a][:, off + m * 128 : off + (m + 1) * 128],
                        xts[a][:, col0 : col0 + 512],
                        start=(a == 0),
                        stop=(a == CA - 1),
                        skip_group_check=True,
                    )

                return go

            def fin():
                nc.vector.tensor_copy(dst[m][:, col0 : col0 + 512], _ps(d, "qkps"))

            d.update(mms=[mk(a) for a in range(CA)], fin=fin)
            return d

        def unit_out(tq, nb):
            d = {"pool": fillp, "tag": "fill", "i": 0}

            def mk(j):
                def go():
                    nc.tensor.matmul(
                        _ps(d, "ops"),
                        hds[j][:, tq * 128 : (tq + 1) * 128],
                        wo_all[:, j * C + nb * 512 : j * C + (nb + 1) * 512],
                        start=(j == 0),
                        stop=(j == 3),
                        skip_group_check=True,
                    )

                return go

            def fin():
                ob = norm.tile([128, 512], F32, tag="ob", name="ob")
                nc.vector.tensor_copy(ob, _ps(d, "ops"))
                eng = nc.sync if (tq * 2 + nb) % 2 == 0 else nc.gpsimd
                eng.dma_start(
                    out=y[tq * 128 : (tq + 1) * 128, nb * 512 : (nb + 1) * 512],
                    in_=ob,
                )

            d.update(mms=[mk(j) for j in range(4)], fin=fin)
            return d

        # ---------------- attention steps ----------------
        # One step = (m, ik, qb) covering BOTH head-halves: the two ST
        # matmuls are emitted back-to-back into disjoint PE row groups
        # (tile_position) so they execute concurrently.
        steps = []
        for m in range(4):
            for qb in range(4):
                for ik in range(4 * qb + 4):
                    steps.append((m, ik, qb))

        LAG = 4
        SLACK = 2

        # First step index needing each input region -> unit deadlines.
        first_need = {}
        for t, (m, ik, qb) in enumerate(steps):
            first_need.setdefault(("kt", m, ik // 8, (ik // 4) % 2), t)
            first_need.setdefault(("qt", m, qb // 2, qb % 2), t)
            first_need.setdefault(("v", ik), t)

        units = []
        for tk in range(NT128):
            d = unit_v(tk)
            d["deadline"] = first_need[("v", tk)] + LAG - 1  # consumed by PV, not ST
            units.append(d)
        for m in range(4):
            for tbp in range(2):
                for hb in range(2):
                    d = unit_qkt(m, 0, qts, tbp, hb)
                    d["deadline"] = first_need.get(("qt", m, tbp, hb), BIG)
                    units.append(d)
                    d = unit_qkt(m, DQ, kts, tbp, hb)
                    d["deadline"] = first_need.get(("kt", m, tbp, hb), BIG)
                    units.append(d)
        for tq in range(NT128):
            for nb in range(2):
                d = unit_out(tq, nb)
                d["deadline"] = BIG
                units.append(d)
        units.sort(key=lambda u: u["deadline"])
        uq = deque(units)
        cur = [None]

        def pump_one():
            if cur[0] is None:
                if not uq:
                    return False
                cur[0] = uq.popleft()
            u = cur[0]
            u["mms"][u["i"]]()
            u["i"] += 1
            if u["i"] == len(u["mms"]):
                u["fin"]()
                cur[0] = None
            return True

        def head_deadline():
            if cur[0] is not None:
                return cur[0]["deadline"]
            return uq[0]["deadline"] if uq else BIG

        # Prolog: the units due before the first step run chunk-major on
        # the (still unused) ST psum ring, so their matmuls interleave
        # with the serial arrival of the input DMA chunks.
        prolog = []
        while uq and uq[0]["deadline"] <= SLACK and len(prolog) < 3:
            d = uq.popleft()
            if len(prolog) < 2:
                d["pool"], d["tag"] = sps, "sps"
            prolog.append(d)
        for a in range(CA):
            for d in prolog:
                d["mms"][a]()
        for d in prolog:
            d["fin"]()

        # ---------------- step emitters ----------------
        staged = {}
        us = {}

        def emit_st_exp(t):
            m, ik, qb = steps[t]
            qstart = qb * 512
            c0 = min(max(128 * ik - qstart, 0), 512)
            kc = slice(ik * 128, (ik + 1) * 128)
            s_ps = sps.tile([128, 1024], F32, tag="sps", name="sps")
            for half in range(2):
                rq = slice(half * 64, half * 64 + 64)
                nc.tensor.matmul(
                    s_ps[:, half * 512 + c0 : half * 512 + 512],
                    kts[m][rq, kc],
                    qts[m][rq, qstart + c0 : qstart + 512],
                    start=True,
                    stop=True,
                )
            p_t = p_pool.tile([128, 1024], BF16, tag="p", name="pt")
            # ONE contiguous activation covers both halves; the [0:c0)
            # columns of each half are stale psum, exp'd but never read.
            nc.scalar.activation(p_t, s_ps, EXP, scale=SCALE)
            if 0 <= 128 * ik - qstart < 512:  # diagonal block
                for half in range(2):
                    lo = half * 512 + c0
                    nc.vector.tensor_mul(p_t[:, lo : lo + 128], p_t[:, lo : lo + 128], tri)
            staged[t] = (p_t, c0)
            return 512 - c0

        def emit_pv(t):
            m, ik, qb = steps[t]
            p_t, c0 = staged.pop(t)
            for half in range(2):
                h = 2 * m + half
                key = (m, half, qb)
                if key not in us:
                    us[key] = u_ps.tile([65, 512], F32, tag="u", name=f"u{half}_{qb}")
                u = us[key]
                nc.tensor.matmul(
                    u[:, c0:512],
                    vts[ik][:, h * 65 : h * 65 + 65],
                    p_t[:, half * 512 + c0 : half * 512 + 512],
                    start=(ik == 0),
                    stop=(ik == 4 * qb + 3),
                    skip_group_check=True,
                )
                if ik == 4 * qb + 3:
                    rq = slice(half * 64, half * 64 + 64)
                    # Evacuate u to SBUF immediately so its bank recycles
                    # without waiting on the full normalization chain.
                    uc = norm.tile([65, 512], F32, tag="uc", name="uc")
                    nc.vector.tensor_copy(uc, u)
                    rec = norm.tile([1, 512], F32, tag="rc", name="rc")
                    nc.vector.reciprocal_approx_fast(out=rec, in_=uc[64:65, :])
                    bc = norm.tile([64, 512], F32, tag="bc", name="bc")
                    nc.gpsimd.partition_broadcast(bc, rec)
                    nc.vector.tensor_mul(
                        hds[m][rq, qb * 512 : (qb + 1) * 512], uc[0:64, :], bc
                    )
                    us.pop(key)

        # ---------------- the global pipeline ----------------
        deficit = 0.0
        for t in range(len(steps) + LAG):
            if t < len(steps):
                while head_deadline() <= t + SLACK:
                    if not pump_one():
                        break
                ncols = emit_st_exp(t)
                deficit += (190.0 + 1.36 * ncols) - (3 * ncols / 2.4 + 60.0)
                while deficit > 0 and pump_one():
                    deficit -= 228.0
            if t >= LAG:
                emit_pv(t - LAG)
        # Drain remaining units (output projection): alternate the psum
        # rings and emit the next unit's matmuls before the previous
        # unit's copy, so copies pipeline behind matmul groups.
        rem = []
        if cur[0] is not None:
            rem.append(cur[0])
            cur[0] = None
        rem.extend(uq)
        uq.clear()
        pend = deque()
        for k, dunit in enumerate(rem):
            dunit["drain"] = True
            if "t" not in dunit and k % 2 == 1:
                dunit["pool"], dunit["tag"] = sps, "sps"
            for f in dunit["mms"][dunit["i"] :]:
                f()
            dunit["i"] = len(dunit["mms"])
            pend.append(dunit)
            if len(pend) >= 2:
                pend.popleft()["fin"]()
        while pend:
            pend.popleft()["fin"]()


def _in_maps(x, w_qkv, w_out):
    maps = []
    for c in range(NCORES):
        b, g = c // 2, c % 2
        h0 = g * DQ
        wqkv = np.concatenate(
            [
                w_qkv[:, h0 : h0 + DQ],
                w_qkv[:, C + h0 : C + h0 + DQ],
                w_qkv[:, 2 * C + h0 : 2 * C + h0 + DQ],
            ],
            axis=1,
        )
        maps.append(
            {
                "xT": np.ascontiguousarray(x[b].T).astype(NP_BF16),
                "wqkv": np.ascontiguousarray(wqkv).astype(NP_BF16),
                "wo": np.ascontiguousarray(
                    w_out[h0 : h0 + DQ, :]
                    .reshape(4, 128, C)
                    .transpose(1, 0, 2)
                    .reshape(128, 4 * C)
                ).astype(NP_BF16),
                "tri": np.triu(np.ones((128, 128), dtype=np.float32)).astype(NP_BF16),
            }
        )
    return maps


def get_bass():
    global _cached
    if _cached is None:
        _cached = _build()
    return _cached


def run(x, w_qkv, w_out, b_out, **spmd_kwargs):
    nc = get_bass()
    res = run_bass_kernel_spmd(
        nc, _in_maps(x, w_qkv, w_out), core_ids=list(range(NCORES)), **spmd_kwargs
    )
    out = np.empty((B, T, C), dtype=np.float32)
    for b in range(B):
        out[b] = res.results[2 * b]["y"] + res.results[2 * b + 1]["y"]
    out += b_out.astype(np.float32)
    return out, res


def kernel(x, w_qkv, w_out, b_out):
    x = np.asarray(x)
    w_qkv = np.asarray(w_qkv)
    w_out = np.asarray(w_out)
    b_out = np.asarray(b_out)
    out, _ = run(x, w_qkv, w_out, b_out)
    return out
